# revision 3
# baseline (speedup 1.0000x reference)
"""Trainium2 Bass kernel for the CSCG batched masked HMM forward pass.

Problem: for each of B=8 padded observation sequences, run a log-space HMM
forward recurrence restricted to 512-state clone blocks selected by
consecutive observation pairs, and read log P(obs) at true_len-1.

Strategy (one sequence per NeuronCore, 8 cores):
  * Work in linear space with exact power-of-two step scaling instead of
    logsumexp: the fp8 table holds f*exp(log_T) (f chosen so the max entry
    sits near 128 and f = 16*2^e_k exactly), and each step multiplies by
    2^-e_k, making the mean per-step growth exactly 1 in expectation - no
    on-device renormalization is needed for 1023 steps (stochastic drift
    is a few nats; bf16 has ~e^+-88 of headroom).
  * The host precomputes the fp8 block-major table once (256 blocks of
    512x512 -> 128 rows x 2KB each, plus one constant pad block), so the
    device never touches the f32 log_T.
  * Per step, one HWDGE dma_start with a register-sourced dynamic DRAM
    offset fetches the contiguous 256KB block for the observation pair -
    no gpsimd SWDGE descriptor generation. DMA issue alternates between
    the SP and ACT sequencers, and the offset registers are loaded in
    batches of 8 to keep both sequencers far off the critical path.
  * The 512x512 matvec runs as 16 PSUM-accumulated (K=128, M=128, N=1)
    matmuls with fp8 weights (FWL fast weight load) whose input/output
    layout is identical ([128 partitions = low 7 bits of state, 4 free =
    high 2 bits]), so the serial chain needs no transposes. The state
    vector lives in four separate [128,1] SBUF tiles so the four
    psum->SBUF column copies are independent and pipeline with the next
    step's matmuls.
  * Steps past true_len-1 multiply by a constant pad block that preserves
    sum(v) exactly, so all cores run a uniform step count and the final
    readout log(sum(v)) equals the value at true_len-1 (up to host-side
    constants).
"""

import math
from contextlib import ExitStack

import numpy as np
import ml_dtypes

N_OBS = 16
C = 512
N_STATES = N_OBS * C  # 8192
B = 8
T = 1024
N_CORES = 8
PAD_BLOCK = N_OBS * N_OBS  # index of the constant pad block
N_TABLE_ROWS = (PAD_BLOCK + 1) * 128  # 33024 rows of 2048 bytes
OFF_BATCH = 8  # offset registers loaded per TENSOR_LOAD


def _build_bass(n_steps: int, k_copy: float, blk_bufs: int = 12):
    import concourse.bass as bass
    import concourse.tile as tile
    from concourse import bacc, mybir

    fp8 = mybir.dt.float8e4
    bf16 = mybir.dt.bfloat16
    f32 = mybir.dt.float32
    i32 = mybir.dt.int32
    SP = mybir.EngineType.SP
    ACT = mybir.EngineType.Activation

    n_e = (n_steps + 1) // 2  # even steps 0,2,...
    n_o = n_steps // 2  # odd steps 1,3,...

    nc = bacc.Bacc(None, target_bir_lowering=False)
    table_in = nc.dram_tensor("table", [N_TABLE_ROWS, 2048], fp8,
                              kind="ExternalInput")
    offs_e_in = nc.dram_tensor("offs_e", [1, max(n_e, 1)], i32,
                               kind="ExternalInput")
    offs_o_in = nc.dram_tensor("offs_o", [1, max(n_o, 1)], i32,
                               kind="ExternalInput")
    v0_in = nc.dram_tensor("v0", [128, 4], bf16, kind="ExternalInput")
    p_out = nc.dram_tensor("p_out", [128, 4], f32, kind="ExternalOutput")

    with ExitStack() as ctx:
        tc = ctx.enter_context(tile.TileContext(nc))

        pconst = ctx.enter_context(tc.tile_pool(name="pconst", bufs=1))
        pblk = ctx.enter_context(tc.tile_pool(name="pblk", bufs=blk_bufs))
        pch = ctx.enter_context(tc.tile_pool(name="pch", bufs=3))
        pfin = ctx.enter_context(tc.tile_pool(name="pfin", bufs=1))
        ps_v = ctx.enter_context(tc.tile_pool(name="ps_v", bufs=2,
                                              space="PSUM"))

        offs_e_sb = pconst.tile([1, max(n_e, 1)], i32, tag="oe")
        nc.sync.dma_start(offs_e_sb[:], offs_e_in[:])
        offs_o_sb = pconst.tile([1, max(n_o, 1)], i32, tag="oo")
        nc.sync.dma_start(offs_o_sb[:], offs_o_in[:])

        v0_sb = pconst.tile([128, 4], bf16, tag="v0")
        nc.sync.dma_start(v0_sb[:], v0_in[:])
        chunks = []
        for i in range(4):
            ci = pch.tile([128, 1], bf16, tag=f"c{i}")
            nc.vector.tensor_copy(ci[:], v0_sb[:, i:i + 1])
            chunks.append(ci)

        def load_batch(which, g0):
            """Load up to OFF_BATCH offsets into registers on one engine."""
            if which == 0:
                sb, n, eng = offs_e_sb, n_e, [SP]
            else:
                sb, n, eng = offs_o_sb, n_o, [ACT]
            gn = min(OFF_BATCH, n - g0)
            _, vals = nc.values_load_multi_w_load_instructions(
                sb[0:1, g0:g0 + gn], engines=eng,
                min_val=0, max_val=(N_TABLE_ROWS - 128),
                skip_runtime_bounds_check=True)
            return list(vals)

        vals_q = [[], []]  # pending offset ScalarValues per parity

        for k in range(n_steps):
            par = k % 2
            if not vals_q[par]:
                vals_q[par] = load_batch(par, k // 2)
            off_val = vals_q[par].pop(0)
            eng = nc.sync if par == 0 else nc.scalar

            blk = pblk.tile([128, 2048], fp8, tag="blk")
            eng.dma_start(blk[:], table_in[bass.ds(off_val, 128), :])

            psum = ps_v.tile([128, 4], f32, tag="v")
            new_chunks = []
            for j in range(4):
                for i in range(4):
                    nc.tensor.matmul(
                        out=psum[:, j:j + 1],
                        lhsT=blk[:, i * 512 + j * 128:
                                 i * 512 + (j + 1) * 128],
                        rhs=chunks[i][:, 0:1],
                        start=(i == 0),
                        stop=(i == 3),
                    )
                cj = pch.tile([128, 1], bf16, tag=f"c{j}")
                nc.vector.tensor_scalar_mul(cj[:], psum[:, j:j + 1], k_copy)
                new_chunks.append(cj)
            chunks = new_chunks

        p_f32 = pfin.tile([128, 4], f32)
        for i in range(4):
            nc.vector.tensor_copy(p_f32[:, i:i + 1], chunks[i][:])
        nc.sync.dma_start(p_out[:], p_f32[:])

    nc.finalize()
    return nc


def _host_prep(log_T, log_pi, obs_batch, true_lens, n_steps):
    """fp8 table, per-core step-offset tables, initial states, constants."""
    fp8_np = ml_dtypes.float8_e4m3

    maxlog = float(np.max(log_T))
    M = math.exp(maxlog)
    # f = 16 * 2^e_k with f*M near 128 => max table entry in [90, 181]
    e_k = int(round(math.log2(128.0 / M) - 4.0))
    assert 0 <= e_k - 9 <= 7, f"pad entry 2^{e_k - 9} not fp8-exact"
    ln_f = math.log(16.0) + e_k * math.log(2.0)
    k_copy = 2.0 ** (-e_k)
    kappa = 2.0 ** (e_k - 9)  # pad entry: rowsum 512*kappa = 2^e_k exactly

    # Block-major fp8 table: row (op*16+oc)*128 + i_lo, col i_hi*512 + j
    # holds f*exp(log_T)[op*512 + i_hi*128 + i_lo, oc*512 + j].
    table = np.empty((N_TABLE_ROWS, 2048), dtype=fp8_np)
    lt = np.asarray(log_T, dtype=np.float32)
    for op in range(N_OBS):
        rows = lt[op * C:(op + 1) * C, :]  # [512, 8192]
        e8 = np.exp(rows + np.float32(ln_f)).astype(fp8_np)
        # [i_hi, i_lo, oc, j] -> [oc, i_lo, i_hi, j]
        e6 = e8.reshape(4, 128, N_OBS, 512).transpose(2, 1, 0, 3)
        table[op * N_OBS * 128:(op + 1) * N_OBS * 128, :] = \
            e6.reshape(N_OBS * 128, 2048)
    table[PAD_BLOCK * 128:, :] = fp8_np(kappa)

    offs = np.empty((N_CORES, 1, max(n_steps, 1)), dtype=np.int32)
    v0 = np.empty((N_CORES, 128, 4), dtype=ml_dtypes.bfloat16)
    host_const = np.empty((N_CORES,), dtype=np.float64)

    for b in range(N_CORES):
        o = np.asarray(obs_batch[b], dtype=np.int64)
        tl = int(true_lens[b])
        blocks = o[:-1] * N_OBS + o[1:]  # step k uses blocks[k]
        blocks = blocks[:n_steps].copy()
        blocks[max(tl - 1, 0):] = PAD_BLOCK
        if n_steps == 0:
            blocks = np.array([PAD_BLOCK], dtype=np.int64)
        offs[b, 0, :] = (blocks * 128).astype(np.int32)

        a0 = np.asarray(log_pi[o[0] * C:(o[0] + 1) * C], dtype=np.float64)
        m0 = float(np.max(a0))
        v0[b] = np.exp(a0 - m0).reshape(4, 128).T.astype(ml_dtypes.bfloat16)
        n_real = min(max(tl - 1, 0), n_steps)  # pad steps preserve sum(v)
        # per real step the kernel multiplies by f*exp(.)*2^-e_k = 16*exp(.)
        host_const[b] = m0 - n_real * math.log(16.0)

    return k_copy, table, offs, v0, host_const


def _run(log_T, log_pi, obs_batch, true_lens, n_steps=None,
         trace=False, blk_bufs=12, **_ignored):
    from concourse.bass_utils import run_bass_kernel_spmd

    log_pi = np.asarray(log_pi, dtype=np.float32)
    obs_batch = np.asarray(obs_batch)
    true_lens = np.asarray(true_lens)
    if n_steps is None:
        n_steps = max(int(np.max(true_lens)) - 1, 0)

    k_copy, table, offs, v0, host_const = _host_prep(
        log_T, log_pi, obs_batch, true_lens, n_steps)

    nc = _build_bass(n_steps, k_copy, blk_bufs)

    in_maps = [
        {"table": table,
         "offs_e": np.ascontiguousarray(offs[b, :, 0::2]),
         "offs_o": np.ascontiguousarray(offs[b, :, 1::2]),
         "v0": np.ascontiguousarray(v0[b])}
        for b in range(N_CORES)
    ]
    res = run_bass_kernel_spmd(nc, in_maps, core_ids=list(range(N_CORES)),
                               trace=trace)
    logZ = np.empty((N_CORES,), dtype=np.float32)
    for b in range(N_CORES):
        p = res.results[b]["p_out"].astype(np.float64)
        logZ[b] = math.log(float(p.sum())) + host_const[b]
    return logZ, res


def kernel(log_T, log_pi, obs_batch, true_lens, n_clones=C, **_ignored):
    assert int(n_clones) == C, f"kernel hardcodes n_clones={C}, got {n_clones}"
    logZ, _ = _run(log_T, log_pi, obs_batch, true_lens)
    return logZ


# revision 4
# speedup vs baseline: 2.2661x; 2.2661x over previous
"""Trainium2 Bass kernel for the CSCG batched masked HMM forward pass.

Problem: for each of B=8 padded observation sequences, run a log-space HMM
forward recurrence restricted to 512-state clone blocks selected by
consecutive observation pairs, and read log P(obs) at true_len-1.

Strategy (one sequence per NeuronCore, 8 cores):
  * Work in linear space with exact power-of-two step scaling instead of
    logsumexp: the fp8 table holds f*exp(log_T) (f chosen so the max entry
    sits near 128 and f = 16*2^e_k exactly), and each step multiplies by
    2^-e_k, making the mean per-step growth exactly 1 in expectation - no
    on-device renormalization is needed for 1023 steps (stochastic drift
    is a few nats; bf16 has ~e^+-88 of headroom).
  * The host precomputes the fp8 block-major table once (256 blocks of
    512x512 -> 128 rows x 2KB each, plus one constant pad block), so the
    device never touches the f32 log_T.
  * Per step, one HWDGE dma_start with a register-sourced dynamic DRAM
    offset fetches the contiguous 256KB block for the observation pair -
    no gpsimd SWDGE descriptor generation. DMA issue alternates between
    the SP and ACT sequencers, and the offset registers are loaded in
    batches of 8 to keep both sequencers far off the critical path.
  * The 512x512 matvec runs as 16 PSUM-accumulated (K=128, M=128, N=1)
    matmuls with fp8 weights (FWL fast weight load) whose input/output
    layout is identical ([128 partitions = low 7 bits of state, 4 free =
    high 2 bits]), so the serial chain needs no transposes. The state
    vector lives in four separate [128,1] SBUF tiles so the four
    psum->SBUF column copies are independent and pipeline with the next
    step's matmuls.
  * Steps past true_len-1 multiply by a constant pad block that preserves
    sum(v) exactly, so all cores run a uniform step count and the final
    readout log(sum(v)) equals the value at true_len-1 (up to host-side
    constants).
"""

import math
from contextlib import ExitStack

import numpy as np
import ml_dtypes

N_OBS = 16
C = 512
N_STATES = N_OBS * C  # 8192
B = 8
T = 1024
N_CORES = 8
PAD_BLOCK = N_OBS * N_OBS  # index of the constant pad block
N_TABLE_ROWS = (PAD_BLOCK + 1) * 128  # 33024 rows of 2048 bytes
OFF_BATCH = 8  # offset registers loaded per TENSOR_LOAD


def _build_bass(n_steps: int, k_copy: float, blk_bufs: int = 12):
    import concourse.bass as bass
    import concourse.tile as tile
    from concourse import bacc, mybir

    fp8 = mybir.dt.float8e4
    bf16 = mybir.dt.bfloat16
    f32 = mybir.dt.float32
    i32 = mybir.dt.int32
    SP = mybir.EngineType.SP
    ACT = mybir.EngineType.Activation

    n_e = (n_steps + 1) // 2  # even steps 0,2,...
    n_o = n_steps // 2  # odd steps 1,3,...

    nc = bacc.Bacc(None, target_bir_lowering=False)
    table_in = nc.dram_tensor("table", [N_TABLE_ROWS, 2048], fp8,
                              kind="ExternalInput")
    offs_e_in = nc.dram_tensor("offs_e", [1, max(n_e, 1)], i32,
                               kind="ExternalInput")
    offs_o_in = nc.dram_tensor("offs_o", [1, max(n_o, 1)], i32,
                               kind="ExternalInput")
    v0_in = nc.dram_tensor("v0", [128, 4], bf16, kind="ExternalInput")
    p_out = nc.dram_tensor("p_out", [128, 4], f32, kind="ExternalOutput")

    with ExitStack() as ctx:
        tc = ctx.enter_context(tile.TileContext(nc))

        pconst = ctx.enter_context(tc.tile_pool(name="pconst", bufs=1))
        pblk = ctx.enter_context(tc.tile_pool(name="pblk", bufs=blk_bufs))
        pch = ctx.enter_context(tc.tile_pool(name="pch", bufs=3))
        pfin = ctx.enter_context(tc.tile_pool(name="pfin", bufs=1))
        ps_v = ctx.enter_context(tc.tile_pool(name="ps_v", bufs=2,
                                              space="PSUM"))

        offs_e_sb = pconst.tile([1, max(n_e, 1)], i32, tag="oe")
        nc.sync.dma_start(offs_e_sb[:], offs_e_in[:])
        offs_o_sb = pconst.tile([1, max(n_o, 1)], i32, tag="oo")
        nc.sync.dma_start(offs_o_sb[:], offs_o_in[:])

        v0_sb = pconst.tile([128, 4], bf16, tag="v0")
        nc.sync.dma_start(v0_sb[:], v0_in[:])
        chunks = []
        for i in range(4):
            ci = pch.tile([128, 1], bf16, tag=f"c{i}")
            nc.vector.tensor_copy(ci[:], v0_sb[:, i:i + 1])
            chunks.append(ci)

        def load_batch(which, g0):
            """Load up to OFF_BATCH offsets into registers on one engine."""
            if which == 0:
                sb, n, eng = offs_e_sb, n_e, [SP]
            else:
                sb, n, eng = offs_o_sb, n_o, [ACT]
            gn = min(OFF_BATCH, n - g0)
            _, vals = nc.values_load_multi_w_load_instructions(
                sb[0:1, g0:g0 + gn], engines=eng,
                min_val=0, max_val=(N_TABLE_ROWS - 128),
                skip_runtime_bounds_check=True)
            return list(vals)

        vals_q = [[], []]  # pending offset ScalarValues per parity

        for k in range(n_steps):
            par = k % 2
            if not vals_q[par]:
                vals_q[par] = load_batch(par, k // 2)
            off_val = vals_q[par].pop(0)
            eng = nc.sync if par == 0 else nc.scalar

            blk = pblk.tile([128, 2048], fp8, tag="blk")
            eng.dma_start(blk[:], table_in[bass.ds(off_val, 128), :])

            new_chunks = []
            for j in range(4):
                psum = ps_v.tile([128, 1], f32, tag=f"v{j}")
                for i in range(4):
                    nc.tensor.matmul(
                        out=psum[:, 0:1],
                        lhsT=blk[:, i * 512 + j * 128:
                                 i * 512 + (j + 1) * 128],
                        rhs=chunks[i][:, 0:1],
                        start=(i == 0),
                        stop=(i == 3),
                    )
                cj = pch.tile([128, 1], bf16, tag=f"c{j}")
                nc.vector.tensor_scalar_mul(cj[:], psum[:, 0:1], k_copy)
                new_chunks.append(cj)
            chunks = new_chunks

        p_f32 = pfin.tile([128, 4], f32)
        for i in range(4):
            nc.vector.tensor_copy(p_f32[:, i:i + 1], chunks[i][:])
        nc.sync.dma_start(p_out[:], p_f32[:])

    nc.finalize()
    return nc


def _host_prep(log_T, log_pi, obs_batch, true_lens, n_steps):
    """fp8 table, per-core step-offset tables, initial states, constants."""
    fp8_np = ml_dtypes.float8_e4m3

    maxlog = float(np.max(log_T))
    M = math.exp(maxlog)
    # f = 16 * 2^e_k with f*M near 128 => max table entry in [90, 181]
    e_k = int(round(math.log2(128.0 / M) - 4.0))
    assert 0 <= e_k - 9 <= 7, f"pad entry 2^{e_k - 9} not fp8-exact"
    ln_f = math.log(16.0) + e_k * math.log(2.0)
    k_copy = 2.0 ** (-e_k)
    kappa = 2.0 ** (e_k - 9)  # pad entry: rowsum 512*kappa = 2^e_k exactly

    # Block-major fp8 table: row (op*16+oc)*128 + i_lo, col i_hi*512 + j
    # holds f*exp(log_T)[op*512 + i_hi*128 + i_lo, oc*512 + j].
    table = np.empty((N_TABLE_ROWS, 2048), dtype=fp8_np)
    lt = np.asarray(log_T, dtype=np.float32)
    for op in range(N_OBS):
        rows = lt[op * C:(op + 1) * C, :]  # [512, 8192]
        e8 = np.exp(rows + np.float32(ln_f)).astype(fp8_np)
        # [i_hi, i_lo, oc, j] -> [oc, i_lo, i_hi, j]
        e6 = e8.reshape(4, 128, N_OBS, 512).transpose(2, 1, 0, 3)
        table[op * N_OBS * 128:(op + 1) * N_OBS * 128, :] = \
            e6.reshape(N_OBS * 128, 2048)
    table[PAD_BLOCK * 128:, :] = fp8_np(kappa)

    offs = np.empty((N_CORES, 1, max(n_steps, 1)), dtype=np.int32)
    v0 = np.empty((N_CORES, 128, 4), dtype=ml_dtypes.bfloat16)
    host_const = np.empty((N_CORES,), dtype=np.float64)

    for b in range(N_CORES):
        o = np.asarray(obs_batch[b], dtype=np.int64)
        tl = int(true_lens[b])
        blocks = o[:-1] * N_OBS + o[1:]  # step k uses blocks[k]
        blocks = blocks[:n_steps].copy()
        blocks[max(tl - 1, 0):] = PAD_BLOCK
        if n_steps == 0:
            blocks = np.array([PAD_BLOCK], dtype=np.int64)
        offs[b, 0, :] = (blocks * 128).astype(np.int32)

        a0 = np.asarray(log_pi[o[0] * C:(o[0] + 1) * C], dtype=np.float64)
        m0 = float(np.max(a0))
        v0[b] = np.exp(a0 - m0).reshape(4, 128).T.astype(ml_dtypes.bfloat16)
        n_real = min(max(tl - 1, 0), n_steps)  # pad steps preserve sum(v)
        # per real step the kernel multiplies by f*exp(.)*2^-e_k = 16*exp(.)
        host_const[b] = m0 - n_real * math.log(16.0)

    return k_copy, table, offs, v0, host_const


def _run(log_T, log_pi, obs_batch, true_lens, n_steps=None,
         trace=False, blk_bufs=12, **_ignored):
    from concourse.bass_utils import run_bass_kernel_spmd

    log_pi = np.asarray(log_pi, dtype=np.float32)
    obs_batch = np.asarray(obs_batch)
    true_lens = np.asarray(true_lens)
    if n_steps is None:
        n_steps = max(int(np.max(true_lens)) - 1, 0)

    k_copy, table, offs, v0, host_const = _host_prep(
        log_T, log_pi, obs_batch, true_lens, n_steps)

    nc = _build_bass(n_steps, k_copy, blk_bufs)

    in_maps = [
        {"table": table,
         "offs_e": np.ascontiguousarray(offs[b, :, 0::2]),
         "offs_o": np.ascontiguousarray(offs[b, :, 1::2]),
         "v0": np.ascontiguousarray(v0[b])}
        for b in range(N_CORES)
    ]
    res = run_bass_kernel_spmd(nc, in_maps, core_ids=list(range(N_CORES)),
                               trace=trace)
    logZ = np.empty((N_CORES,), dtype=np.float32)
    for b in range(N_CORES):
        p = res.results[b]["p_out"].astype(np.float64)
        logZ[b] = math.log(float(p.sum())) + host_const[b]
    return logZ, res


def kernel(log_T, log_pi, obs_batch, true_lens, n_clones=C, **_ignored):
    assert int(n_clones) == C, f"kernel hardcodes n_clones={C}, got {n_clones}"
    logZ, _ = _run(log_T, log_pi, obs_batch, true_lens)
    return logZ
